# revision 1
# baseline (speedup 1.0000x reference)
"""Trainium2 (8 NeuronCores, SPMD) kernel for a 4-layer GCN + mean-pool + FC head.

Strategy (nodes dst-sharded contiguously across 8 cores; edges binned by
(dst-window of 128, src-chunk of 25000) so int16 gather indices work):

Per conv layer (one SPMD launch, same compiled program for all 4 layers):
  xt = dinv * x (bf16, node-major, full replica in each core's HBM)
  M[:, d]  = sum_{e: dst[e]=d} xt[:, src[e]] + 2*xt[:, d]
  xo[:, d] = relu(W^T (dinv[d] * M[:, d]) + b)          (feature-major bf16)

Device mechanics per 128-dst window:
  - dma_gather (SWDGE, 4 queues, <=1024 idxs/call) pulls edge-source rows
    into edge-major [128e, 128f] bf16 tiles.
  - S tiles [e, d] = is_equal(iota_row, dstloc) built on DVE (batched).
  - PE matmuls (lhsT=G, rhs=S) accumulate M feature-major in PSUM fp32.
  - Self loop: plain DMA of the core's own 128 rows + matmul with const 2*I.
  - DVE evacuates PSUM multiplying by a broadcast dinv row (bf16),
    then W matmul + bias/ReLU (ACT) produce the bf16 output block.

Host (numpy): deg/dinv, edge binning, inter-layer transpose + dinv scale,
final mean-pool (segment reduceat over sorted batch) and the tiny FC head.
"""
import contextlib
import ctypes
import sys
import types

import numpy as np
import ml_dtypes

import concourse.bass as bass
import concourse.bacc as bacc
import concourse.mybir as mybir
import concourse.tile as tile

BF16 = mybir.dt.bfloat16
F32 = mybir.dt.float32
I16 = mybir.dt.int16
AF = mybir.ActivationFunctionType
NPBF16 = ml_dtypes.bfloat16

P = 128
PAD_DSTLOC = 200.0  # sentinel dst-local id for padding edges (is_equal -> 0)

N_NODES = 100000
N_CORES = 8
N_CONVS = 4
CHUNKS = 4
WGW = 6  # windows per gather group
SBATCH = 8  # tiles per is_equal op
NQUEUES = 4  # SWDGE queues for gathers
GMAX = 8  # tiles per dma_gather call (1024 idxs — ucode cap)
BUFS_G = 8
BUFS_S = 8
BUFS_IDX = 8

NPC = N_NODES // N_CORES  # 12500
NWIN = (NPC + P - 1) // P  # 98
NPC_PAD = NWIN * P  # 12544
CHR = N_NODES // CHUNKS  # 25000


# ---------------------------------------------------------------------------
# axon NTFF profile hook (this image's antenv lacks axon_hooks; recreate it so
# run_bass_kernel_spmd(trace=True) can report HW exec time)
# ---------------------------------------------------------------------------
def _install_profile_shim():
    if "antenv.axon_hooks" in sys.modules:
        return
    so_path = "/opt/axon/libaxon_pjrt.so"

    def _ntff_profile_via_ctypes(path):
        try:
            lib = ctypes.CDLL(path)
        except OSError:
            return None
        if not hasattr(lib, "axon_start_nrt_profile"):
            return None
        lib.axon_start_nrt_profile.argtypes = [
            ctypes.POINTER(ctypes.c_int64),
            ctypes.c_size_t,
        ]
        lib.axon_start_nrt_profile.restype = ctypes.c_int64
        lib.axon_stop_nrt_profile.argtypes = [ctypes.c_char_p]
        lib.axon_stop_nrt_profile.restype = ctypes.c_int64

        @contextlib.contextmanager
        def _hook(output_dir, device_ids):
            import jax

            jax.devices()
            if device_ids:
                ids = (ctypes.c_int64 * len(device_ids))(*device_ids)
                rc = lib.axon_start_nrt_profile(ids, len(device_ids))
            else:
                rc = lib.axon_start_nrt_profile(None, 0)
            if rc != 0:
                raise RuntimeError(f"axon_start_nrt_profile rc={rc}")
            try:
                yield
            finally:
                n = lib.axon_stop_nrt_profile(str(output_dir).encode())
                if n < 0:
                    raise RuntimeError(f"axon_stop_nrt_profile rc={n}")

        return _hook

    mod = types.ModuleType("antenv.axon_hooks")
    hook = _ntff_profile_via_ctypes(so_path)
    mod.get_axon_ntff_profile_hook = lambda: hook
    mod.set_axon_ntff_profile_hook = lambda h: None
    try:
        import antenv

        antenv.axon_hooks = mod
    except ImportError:
        pass
    sys.modules["antenv.axon_hooks"] = mod


_install_profile_shim()

from concourse.bass_utils import run_bass_kernel_spmd  # noqa: E402


# ---------------------------------------------------------------------------
# host-side edge preprocessing
# ---------------------------------------------------------------------------
def _host_prep(src, dst):
    """Bin edges by (core, window, chunk); per-core idx/dstloc arrays.

    Bin sizes are padded to the max over cores and to tile multiples so one
    SPMD program fits all cores; padding edges use idx 0 with a sentinel
    dst-local id that zeroes their S column.
    """
    core = dst // NPC
    drem = dst % NPC
    win = drem // P
    dloc = drem - win * P
    chunk = src // CHR
    srcloc = (src - chunk * CHR).astype(np.int64)
    assert srcloc.max() < 32768

    binid = ((core * NWIN + win) * CHUNKS + chunk).astype(np.int64)
    counts = np.bincount(binid, minlength=N_CORES * NWIN * CHUNKS).reshape(
        N_CORES, NWIN, CHUNKS
    )
    tiles = -(-counts.max(axis=0) // P)  # [NWIN, CHUNKS] ceil
    wgs = [list(range(w0, min(w0 + WGW, NWIN))) for w0 in range(0, NWIN, WGW)]

    slots = tiles * P
    bin_start = np.zeros((NWIN, CHUNKS), np.int64)
    off = 0
    for wg in wgs:
        for c in range(CHUNKS):
            for w in wg:
                bin_start[w, c] = off
                off += slots[w, c]
    L = off
    nt_total = int(tiles.sum())
    assert L == nt_total * P

    order = np.argsort(binid, kind="stable")
    sorted_bin = binid[order]
    uniq, first_idx = np.unique(sorted_bin, return_index=True)
    start_of_bin = np.zeros(N_CORES * NWIN * CHUNKS, np.int64)
    start_of_bin[uniq] = first_idx
    within = np.arange(len(order)) - start_of_bin[sorted_bin]

    s_core = core[order]
    pos = bin_start[win[order], chunk[order]] + within

    per_core = []
    for cc in range(N_CORES):
        m = s_core == cc
        idx_arr = np.zeros(L, np.int64)
        dst_arr = np.full(L, PAD_DSTLOC, np.float32)
        idx_arr[pos[m]] = srcloc[order][m]
        dst_arr[pos[m]] = dloc[order][m]
        idx16 = idx_arr.reshape(L // 16, 16).T.astype(np.int16)  # [16, L//16]
        idx128 = np.tile(idx16, (8, 1))
        dstloc = dst_arr.reshape(nt_total, P).T.astype(NPBF16)
        per_core.append({"idx": idx128, "dstloc": dstloc})
    return tiles, wgs, nt_total, per_core


# ---------------------------------------------------------------------------
# device program (one conv layer; same program reused for all 4 launches)
# ---------------------------------------------------------------------------
def _build_program(tiles, wgs, nt_total):
    NT = nt_total
    L = NT * P

    nc = bacc.Bacc(
        "TRN2", target_bir_lowering=False, debug=False, num_swdge_queues=NQUEUES
    )
    xt = nc.dram_tensor("xt", [N_NODES, P], BF16, kind="ExternalInput")
    xt_own = nc.dram_tensor("xt_own", [NPC_PAD, P], BF16, kind="ExternalInput")
    idx_in = nc.dram_tensor("idx", [P, L // 16], I16, kind="ExternalInput")
    dstloc_in = nc.dram_tensor("dstloc", [P, NT], BF16, kind="ExternalInput")
    w_in = nc.dram_tensor("wmat", [P, P], BF16, kind="ExternalInput")
    b_in = nc.dram_tensor("bias", [P, 1], F32, kind="ExternalInput")
    dinvr_in = nc.dram_tensor("dinv_row", [P, NPC_PAD], BF16, kind="ExternalInput")
    iota_in = nc.dram_tensor("iota", [P, P], BF16, kind="ExternalInput")
    s2i_in = nc.dram_tensor("s2i", [P, P], BF16, kind="ExternalInput")
    xo = nc.dram_tensor("xo", [P, NPC_PAD], BF16, kind="ExternalOutput")

    with tile.TileContext(nc) as tc:
        with (
            tc.tile_pool(name="const", bufs=1) as cpool,
            tc.tile_pool(name="idx", bufs=BUFS_IDX) as ipool,
            tc.tile_pool(name="g", bufs=BUFS_G) as gpool,
            tc.tile_pool(name="s", bufs=BUFS_S) as spool,
            tc.tile_pool(name="selfp", bufs=4) as selfpool,
            tc.tile_pool(name="m", bufs=3) as mpool,
            tc.tile_pool(name="xop", bufs=3) as xopool,
            tc.tile_pool(name="psm", bufs=3, space="PSUM") as psm_pool,
            tc.tile_pool(name="psh", bufs=2, space="PSUM") as psh_pool,
        ):
            dst_t = cpool.tile([P, NT], BF16)
            nc.sync.dma_start(dst_t[:], dstloc_in[:])
            dinvr_t = cpool.tile([P, NPC_PAD], BF16)
            nc.sync.dma_start(dinvr_t[:], dinvr_in[:])
            iota_t = cpool.tile([P, P], BF16)
            nc.sync.dma_start(iota_t[:], iota_in[:])
            s2i_t = cpool.tile([P, P], BF16)
            nc.sync.dma_start(s2i_t[:], s2i_in[:])
            w_t = cpool.tile([P, P], BF16)
            nc.sync.dma_start(w_t[:], w_in[:])
            b_t = cpool.tile([P, 1], F32)
            nc.sync.dma_start(b_t[:], b_in[:])

            col = 0
            tg = 0
            qctr = 0
            for wg in wgs:
                slabs = []
                for c in range(CHUNKS):
                    T = int(sum(tiles[w, c] for w in wg))
                    if T == 0:
                        slabs.append(None)
                        continue
                    cols = T * P // 16
                    it = ipool.tile([P, cols], I16, tag="idx")
                    nc.sync.dma_start(it[:], idx_in[:, col : col + cols])
                    col += cols
                    g = gpool.tile([P, T, P], BF16, tag="g")
                    for t0 in range(0, T, GMAX):
                        nt = min(GMAX, T - t0)
                        nc.gpsimd.dma_gather(
                            g[:, t0 : t0 + nt, :],
                            xt[c * CHR : (c + 1) * CHR, :],
                            it[:, t0 * 8 : (t0 + nt) * 8],
                            nt * P,
                            nt * P,
                            P,
                            queue_num=qctr % NQUEUES,
                        )
                        qctr += 1
                    s = spool.tile([P, T, P], BF16, tag="s")
                    for b0 in range(0, T, SBATCH):
                        nb = min(SBATCH, T - b0)
                        nc.vector.tensor_tensor(
                            s[:, b0 : b0 + nb, :],
                            iota_t[:, None, :].to_broadcast([P, nb, P]),
                            dst_t[:, tg + b0 : tg + b0 + nb, None].to_broadcast(
                                [P, nb, P]
                            ),
                            mybir.AluOpType.is_equal,
                        )
                    offs = {}
                    o = 0
                    for w in wg:
                        offs[w] = o
                        o += int(tiles[w, c])
                    slabs.append((g, s, offs))
                    tg += T

                for w in wg:
                    ps = psm_pool.tile([P, P], F32, tag="psm")
                    first = True
                    for c in range(CHUNKS):
                        if slabs[c] is None:
                            continue
                        g, s, offs = slabs[c]
                        for t in range(int(tiles[w, c])):
                            o = offs[w] + t
                            nc.tensor.matmul(
                                ps[:], g[:, o, :], s[:, o, :], start=first, stop=False
                            )
                            first = False
                    gs = selfpool.tile([P, P], BF16, tag="gself")
                    nc.sync.dma_start(gs[:], xt_own[w * P : (w + 1) * P, :])
                    nc.tensor.matmul(ps[:], gs[:], s2i_t[:], start=first, stop=True)

                    m = mpool.tile([P, P], BF16, tag="m")
                    nc.vector.tensor_tensor(
                        m[:], ps[:], dinvr_t[:, w * P : (w + 1) * P],
                        mybir.AluOpType.mult,
                    )
                    ph = psh_pool.tile([P, P], F32, tag="psh")
                    nc.tensor.matmul(ph[:], w_t[:], m[:], start=True, stop=True)
                    xo_sb = xopool.tile([P, P], BF16, tag="xo")
                    nc.scalar.activation(xo_sb[:], ph[:], AF.Relu, bias=b_t[:])
                    nc.sync.dma_start(xo[:, w * P : (w + 1) * P], xo_sb[:])
    nc.compile()
    return nc


_CACHE = {}


def _get_program(src, dst):
    key = (hash(src.tobytes()) ^ hash(dst.tobytes()), len(src))
    if key not in _CACHE:
        tiles, wgs, nt_total, per_core = _host_prep(src, dst)
        nc = _build_program(tiles, wgs, nt_total)
        _CACHE.clear()
        _CACHE[key] = (nc, per_core)
    return _CACHE[key]


def kernel(
    x,
    edge_index,
    batch,
    batch_size,
    conv_w,
    conv_b,
    fc1_w,
    fc1_b,
    fc2_w,
    fc2_b,
    profile=False,
):
    x = np.asarray(x, np.float32)
    edge_index = np.asarray(edge_index, np.int64)
    batch = np.asarray(batch, np.int64)
    conv_w = np.asarray(conv_w, np.float32)
    conv_b = np.asarray(conv_b, np.float32)
    G = int(batch_size)
    n = x.shape[0]
    assert n == N_NODES and edge_index.shape[0] == 2

    src, dst = edge_index[0], edge_index[1]
    deg = np.bincount(dst, minlength=n).astype(np.float32) + 2.0
    dinv = (1.0 / np.sqrt(deg)).astype(np.float32)

    nc, per_core = _get_program(src, dst)

    iota = np.tile(np.arange(P, dtype=np.float32), (P, 1)).astype(NPBF16)
    s2i = (2.0 * np.eye(P, dtype=np.float32)).astype(NPBF16)
    dinv_rows = []
    for c in range(N_CORES):
        dr = np.zeros(NPC_PAD, np.float32)
        dr[:NPC] = dinv[c * NPC : (c + 1) * NPC]
        dinv_rows.append(np.tile(dr[None, :], (P, 1)).astype(NPBF16))

    xt = (dinv[:, None] * x).astype(NPBF16)
    total_ns = 0
    for layer in range(N_CONVS):
        wmat_bf = conv_w[layer].astype(NPBF16)
        bias_f = conv_b[layer].astype(np.float32).reshape(P, 1)
        maps = []
        for c in range(N_CORES):
            own = np.zeros((NPC_PAD, P), NPBF16)
            own[:NPC] = xt[c * NPC : (c + 1) * NPC]
            maps.append(
                {
                    "xt": xt,
                    "xt_own": own,
                    "idx": per_core[c]["idx"],
                    "dstloc": per_core[c]["dstloc"],
                    "wmat": wmat_bf,
                    "bias": bias_f,
                    "dinv_row": dinv_rows[c],
                    "iota": iota,
                    "s2i": s2i,
                }
            )
        res = run_bass_kernel_spmd(
            nc, maps, core_ids=list(range(N_CORES)), trace=profile
        )
        if profile and res.exec_time_ns is not None:
            total_ns += int(res.exec_time_ns)
        xp = np.empty((n, P), np.float32)
        for c in range(N_CORES):
            blk = res.results[c]["xo"].astype(np.float32).T
            xp[c * NPC : (c + 1) * NPC] = blk[:NPC]
        if layer < N_CONVS - 1:
            xt = (dinv[:, None] * xp).astype(NPBF16)

    starts = np.searchsorted(batch, np.arange(G))
    sums = np.add.reduceat(xp, starts, axis=0)
    cnt = np.bincount(batch, minlength=G).astype(np.float32)
    sums[cnt == 0] = 0.0
    pooled = sums / np.maximum(cnt, 1.0)[:, None]
    h = np.maximum(
        pooled @ np.asarray(fc1_w, np.float32) + np.asarray(fc1_b, np.float32), 0.0
    )
    out = h @ np.asarray(fc2_w, np.float32) + np.asarray(fc2_b, np.float32)
    if profile:
        print(f"HW exec time: {total_ns} ns")
    return out[:, 0].astype(np.float32)
